# revision 6
# baseline (speedup 1.0000x reference)
"""ChunkCrossAttention Trainium2 kernel (v3: chunked AllGather-KV pipeline).

Math (per reference):
  x = chunk_embeddings[0]                      # (S, L)
  k, v = split(x @ W_kv.T)                     # (S, D) each
  scores = einsum('jqd,sd->jqs', q, k) / sqrt(D), masked
  attn = softmax(scores, -1)
  out = (attn @ v) @ W_out.T + q  -> LayerNorm(gamma, beta)

Strategy (8 NeuronCores):
  - KV projection sharded over S: each core projects its own 512 keys
    in 4 chunks of 128, fp8 DoubleRow matmuls (wkvT resident in SBUF,
    xT streamed). After each chunk the k^T/v' fp8 payload is
    AllGathered; the 4 AGs pipeline behind each other and behind the
    projection, so foreign K/V arrives while the PE is still busy.
  - W_out folded into v (v' = v @ W_out.T scaled, plus a ones column
    that makes the attention matmul emit the softmax denominator).
  - Attention is query-sharded: each core handles its own 1024 query
    rows over all 4096 keys (exp without max-subtraction, shifted -3
    to stay in fp8e4 range; mask folded into the Exp bias). Scores run
    fp8 DoubleRow; attn@v' accumulates over the 32 key tiles in PSUM.
    The attn@v' matmuls for key-tile-pair n are issued after the score
    matmuls of pair n+1, so the PE never stalls behind the Exp
    activations (software pipelining).
  - Epilogue (denominator divide, residual, LayerNorm) per 128-row
    q tile; rsqrt computed on the vector engine (fast-inverse-sqrt +
    2 Newton steps) so the scalar engine keeps its Exp table.
  - fp8 scaling: W_kv pre-scaled x64 host-side (its std 1/64 would
    land in fp8e4's subnormal range), divided back out during the
    PSUM->fp8 copies; W_out likewise x16.
"""
import sys

sys.path.insert(0, "/opt/trn_rl_repo")

import numpy as np

import concourse.bacc as bacc
import concourse.mybir as mybir
import concourse.tile as tile
from concourse.bass_utils import run_bass_kernel_spmd

N_CORES = 8
J, Q, D = 64, 128, 256
S, L = 4096, 4096
S_LOC = S // N_CORES          # 512 keys per core
QR = (J // N_CORES) * Q       # 1024 query rows per core
CH = 4                        # phase-1 key chunks per core
CK = S_LOC // CH              # 128 keys per chunk
DP = 272                      # at free dim: 256 outputs + denom + pad to x16
N_ST = S // 128               # 32 key tiles
LN_EPS = 1e-5
SCALE = 1.0 / np.sqrt(D)
EXP_SHIFT = -3.0              # folded into maskb host-side
KV_SCALE = 64.0               # host premultiplies wkvT
WO_SCALE = 16.0               # host premultiplies woutT
AT_DR = False                 # DoubleRow for attn@v' matmuls

F32 = mybir.dt.float32
I32 = mybir.dt.int32
FP8 = mybir.dt.float8e4
AF = mybir.ActivationFunctionType
ALU = mybir.AluOpType
DR = mybir.MatmulPerfMode.DoubleRow

RSQRT_MAGIC = 0x5f3759df


def build_program():
    nc = bacc.Bacc(None, num_devices=N_CORES)

    xT = nc.declare_dram_parameter("xT", [L, S_LOC], FP8, isOutput=False)
    wkvT = nc.declare_dram_parameter("wkvT", [L, 2 * D], FP8, isOutput=False)
    qT = nc.declare_dram_parameter("qT", [D, QR], FP8, isOutput=False)
    qres = nc.declare_dram_parameter("qres", [QR, D], F32, isOutput=False)
    woutT = nc.declare_dram_parameter("woutT", [D, D], FP8, isOutput=False)
    maskb = nc.declare_dram_parameter("maskb", [128, N_ST], F32, isOutput=False)
    gamma = nc.declare_dram_parameter("gamma", [D], F32, isOutput=False)
    beta = nc.declare_dram_parameter("beta", [D], F32, isOutput=False)
    y = nc.declare_dram_parameter("y", [QR, D], F32, isOutput=True)

    # per-chunk AllGather payload: kT (2*CK) + vp (DP) fp8 per partition
    AGW = 2 * CK + DP         # 528
    ag_in = [nc.dram_tensor(f"ag_in{c}", [128, AGW], FP8) for c in range(CH)]
    ag_out = [nc.dram_tensor(f"ag_out{c}", [N_CORES, 128, AGW], FP8,
                             addr_space="Shared") for c in range(CH)]

    import concourse.bass as bass

    with tile.TileContext(nc) as tc:
        with tc.tile_pool(name="singles", bufs=1) as singles, \
             tc.tile_pool(name="xw", bufs=3) as xw, \
             tc.tile_pool(name="kv", bufs=1) as kvp, \
             tc.tile_pool(name="exp", bufs=3) as epool, \
             tc.tile_pool(name="small", bufs=8) as small:

            # ---- resident inputs (wkv split in 4 so matmuls start early) ----
            wkv_sb = singles.tile([128, L // 128, 2 * D], FP8)
            for w4 in range(4):
                nc.sync.dma_start(
                    out=wkv_sb[:, w4 * 8:(w4 + 1) * 8, :],
                    in_=wkvT[w4 * 1024:(w4 + 1) * 1024, :].rearrange(
                        "(a p) e -> p a e", p=128))
            qT_sb = singles.tile([128, 2, QR], FP8)
            nc.gpsimd.dma_start(
                out=qT_sb, in_=qT.rearrange("(dc p) q -> p dc q", p=128))
            woutT_sb = singles.tile([128, 2, D], FP8)
            nc.gpsimd.dma_start(
                out=woutT_sb, in_=woutT.rearrange("(dc p) e -> p dc e", p=128))
            maskb_sb = singles.tile([128, N_ST], F32)
            nc.gpsimd.dma_start(out=maskb_sb, in_=maskb[:, :])
            qres_sb = singles.tile([128, QR // 128, D], F32)
            nc.gpsimd.dma_start(
                out=qres_sb, in_=qres.rearrange("(t p) d -> p t d", p=128))
            g_ap = gamma[:]
            gamma_sb = singles.tile([128, D], F32)
            nc.gpsimd.dma_start(out=gamma_sb, in_=bass.AP(
                tensor=g_ap.tensor, offset=g_ap.offset,
                ap=[[0, 128], g_ap.ap[0]]))
            b_ap = beta[:]
            beta_sb = singles.tile([128, D], F32)
            nc.gpsimd.dma_start(out=beta_sb, in_=bass.AP(
                tensor=b_ap.tensor, offset=b_ap.offset,
                ap=[[0, 128], b_ap.ap[0]]))

            kT_all = singles.tile([128, 2, S], FP8)
            vp_all = singles.tile([128, N_ST, DP], FP8)

            # ---- phase 1: project local keys, chunk by chunk; AG each ----
            ps1 = tc.tile_pool(name="ps_kv", bufs=1, space="PSUM")
            ps_kv = ps1.__enter__()
            for c in range(CH):
                acc = [ps_kv.tile([128, 512], F32, tag=f"acc{h}",
                                  name=f"acc{c}_{h}") for h in range(4)]
                for lb in range(L // 512):
                    xt = xw.tile([128, 4, CK], FP8, tag="xt")
                    nc.sync.dma_start(
                        out=xt,
                        in_=xT[lb * 512:(lb + 1) * 512,
                               c * CK:(c + 1) * CK].rearrange(
                            "(a p) s -> p a s", p=128))
                    for ap_ in range(2):
                        A = lb * 4 + 2 * ap_
                        first = lb == 0 and ap_ == 0
                        last = lb == L // 512 - 1 and ap_ == 1
                        for h in range(4):
                            nc.tensor.matmul(
                                acc[h][:, 0:CK],
                                wkv_sb[:, A:A + 2, h * 128:(h + 1) * 128],
                                xt[:, 2 * ap_:2 * ap_ + 2, :],
                                start=first, stop=last, perf_mode=DR)

                # quantize k^T, v^T to fp8 (undo the x64 W_kv prescale)
                kt_c = kvp.tile([128, 2, CK], FP8, name=f"ktc{c}")
                vt_c = kvp.tile([128, 2, CK], FP8, name=f"vtc{c}")
                for dc in range(2):
                    nc.scalar.activation(out=kt_c[:, dc, :], in_=acc[dc][:, 0:CK],
                                         func=AF.Copy, scale=1.0 / KV_SCALE)
                    nc.scalar.activation(out=vt_c[:, dc, :],
                                         in_=acc[2 + dc][:, 0:CK],
                                         func=AF.Copy, scale=1.0 / KV_SCALE)
                # v' = v @ W_out.T (scaled), plus ones column at DP col 256
                vp_c = kvp.tile([128, DP], FP8, name=f"vpc{c}")
                nc.vector.memset(vp_c, 0.0)
                pv = ps_kv.tile([128, 512], F32, tag="pv", name=f"pv{c}")
                nc.tensor.matmul(pv[:, 0:D], vt_c, woutT_sb,
                                 start=True, stop=True, perf_mode=DR)
                nc.scalar.activation(out=vp_c[:, 0:D], in_=pv[:, 0:D],
                                     func=AF.Copy, scale=1.0 / WO_SCALE)
                nc.vector.memset(vp_c[:, D:D + 1], 1.0)

                # ship local chunk, gather everyone's
                nc.sync.dma_start(
                    out=ag_in[c][:, 0:2 * CK].rearrange(
                        "p (dc s) -> p dc s", dc=2),
                    in_=kt_c)
                nc.sync.dma_start(out=ag_in[c][:, 2 * CK:AGW], in_=vp_c)
                nc.gpsimd.collective_compute(
                    "AllGather", ALU.bypass,
                    replica_groups=[list(range(N_CORES))],
                    ins=[ag_in[c][:, :]], outs=[ag_out[c][:, :, :]])
                for r in range(N_CORES):
                    slot = c * N_CORES + r
                    nc.sync.dma_start(
                        out=kT_all[:, :, slot * CK:(slot + 1) * CK],
                        in_=ag_out[c][r, :, 0:2 * CK].rearrange(
                            "p (dc s) -> p dc s", dc=2))
                    nc.sync.dma_start(
                        out=vp_all[:, slot, :],
                        in_=ag_out[c][r, :, 2 * CK:AGW])
            ps1.__exit__(None, None, None)

            # ---- phase 2: q-sharded attention over all keys ----
            ps2 = tc.tile_pool(name="ps_at", bufs=1, space="PSUM")
            ps_at = ps2.__enter__()
            ps3 = tc.tile_pool(name="ps_sc", bufs=3, space="PSUM")
            ps_sc = ps3.__enter__()

            NP = N_ST // 2        # 16 key-tile pairs

            def emit_at(at, stp, ex2):
                for qt in range(4):
                    if AT_DR:
                        nc.tensor.matmul(
                            at[qt][:, 0:DP],
                            ex2[:, :, qt * 128:(qt + 1) * 128],
                            vp_all[:, stp * 2:stp * 2 + 2, :],
                            start=(stp == 0), stop=(stp == NP - 1),
                            perf_mode=DR)
                    else:
                        for par in range(2):
                            nc.tensor.matmul(
                                at[qt][:, 0:DP],
                                ex2[:, par, qt * 128:(qt + 1) * 128],
                                vp_all[:, stp * 2 + par, :],
                                start=(stp == 0 and par == 0),
                                stop=(stp == NP - 1 and par == 1))

            y_r = y.rearrange("(hh t p) d -> hh p t d", hh=2, p=128)
            for half in range(2):
                at = [ps_at.tile([128, 512], F32, tag=f"at{qt}",
                                 name=f"at{half}_{qt}") for qt in range(4)]
                prev = None
                for stp in range(NP):
                    ex2 = epool.tile([128, 2, 512], FP8, tag="ex")
                    for par in range(2):
                        st = stp * 2 + par
                        sc = ps_sc.tile([128, 512], F32, tag="sc")
                        nc.tensor.matmul(
                            sc, kT_all[:, :, st * 128:(st + 1) * 128],
                            qT_sb[:, :, half * 512:(half + 1) * 512],
                            start=True, stop=True, perf_mode=DR)
                        nc.scalar.activation(out=ex2[:, par, :], in_=sc,
                                             func=AF.Exp,
                                             bias=maskb_sb[:, st:st + 1],
                                             scale=SCALE)
                    if prev is not None:
                        emit_at(at, stp - 1, prev)
                    prev = ex2
                emit_at(at, NP - 1, prev)

                # ---- epilogue: denom divide, residual, LayerNorm ----
                h_half = singles.tile([128, 4, D], F32, name=f"h_half{half}")
                vars4 = small.tile([128, 4], F32, tag="vars4")
                mus4 = small.tile([128, 4], F32, tag="mus4")
                for qt in range(4):
                    t = 4 * half + qt
                    hs = h_half[:, qt, :]
                    rec = small.tile([128, 1], F32, tag="rec")
                    nc.vector.reciprocal(out=rec, in_=at[qt][:, D:D + 1])
                    nc.vector.tensor_scalar_mul(out=hs, in0=at[qt][:, 0:D],
                                                scalar1=rec)
                    nc.vector.tensor_add(out=hs, in0=hs, in1=qres_sb[:, t, :])
                    stats = small.tile([128, 6], F32, tag="stats")
                    nc.vector.bn_stats(out=stats, in_=hs)
                    mv = small.tile([128, 2], F32, tag="mv")
                    nc.vector.bn_aggr(out=mv, in_=stats)
                    nc.vector.tensor_copy(out=mus4[:, qt:qt + 1], in_=mv[:, 0:1])
                    nc.vector.tensor_copy(out=vars4[:, qt:qt + 1], in_=mv[:, 1:2])
                # rstd = 1/sqrt(var+eps) on DVE: fast-inv-sqrt + 2 Newton steps
                veps = small.tile([128, 4], F32, tag="veps")
                nc.vector.tensor_scalar(out=veps, in0=vars4, scalar1=1.0,
                                        scalar2=LN_EPS, op0=ALU.mult,
                                        op1=ALU.add)
                yv = small.tile([128, 4], F32, tag="yv")
                yv_i = yv.bitcast(I32)
                nc.vector.tensor_scalar(out=yv_i, in0=veps.bitcast(I32),
                                        scalar1=1, scalar2=None,
                                        op0=ALU.logical_shift_right)
                nc.vector.tensor_scalar(out=yv_i, in0=yv_i,
                                        scalar1=RSQRT_MAGIC, scalar2=-1,
                                        op0=ALU.subtract, op1=ALU.mult)
                tn = small.tile([128, 4], F32, tag="tn")
                for _ in range(2):
                    nc.vector.tensor_mul(out=tn, in0=yv, in1=yv)
                    nc.vector.tensor_mul(out=tn, in0=tn, in1=veps)
                    nc.vector.tensor_scalar(out=tn, in0=tn, scalar1=-0.5,
                                            scalar2=1.5, op0=ALU.mult,
                                            op1=ALU.add)
                    nc.vector.tensor_mul(out=yv, in0=yv, in1=tn)
                for qt in range(4):
                    hs = h_half[:, qt, :]
                    nc.vector.tensor_scalar(out=hs, in0=hs,
                                            scalar1=mus4[:, qt:qt + 1],
                                            scalar2=yv[:, qt:qt + 1],
                                            op0=ALU.subtract, op1=ALU.mult)
                    nc.vector.tensor_mul(out=hs, in0=hs, in1=gamma_sb)
                    nc.vector.tensor_add(out=hs, in0=hs, in1=beta_sb)
                nc.gpsimd.dma_start(out=y_r[half], in_=h_half)

            ps3.__exit__(None, None, None)
            ps2.__exit__(None, None, None)

    nc.finalize()
    return nc


_NC_CACHE = None


def _make_in_maps(inputs):
    jq = np.asarray(inputs["justice_queries"], dtype=np.float32)
    x = np.asarray(inputs["chunk_embeddings"], dtype=np.float32)[0]
    mask = np.asarray(inputs["chunk_mask"])
    wkv = np.asarray(inputs["W_kv"], dtype=np.float32)
    wout = np.asarray(inputs["W_out"], dtype=np.float32)
    gamma = np.asarray(inputs["ln_gamma"], dtype=np.float32)
    beta = np.asarray(inputs["ln_beta"], dtype=np.float32)

    import ml_dtypes
    fp8 = ml_dtypes.float8_e4m3
    xT = np.ascontiguousarray(x.T.astype(fp8))                    # (L, S)
    wkvT = np.ascontiguousarray((wkv.T * KV_SCALE).astype(fp8))   # (L, 2D)
    flat = np.ascontiguousarray(jq.reshape(J * Q, D))             # (8192, D)
    qTf = flat.T.astype(fp8)                                      # (D, 8192)
    woutT = np.ascontiguousarray((wout.T * WO_SCALE).astype(fp8))  # (D, D)

    # mask bias in the AllGather key order: s = c*(8*CK) + r*CK + sloc
    # maps to original key r*S_LOC + c*CK + sloc; -3 shift keeps exp in
    # fp8e4 range (ratios cancel via the denominator).
    bias_orig = np.where(mask != 0, 0.0, -1e30).astype(np.float32) + EXP_SHIFT
    sidx = np.arange(S)
    c_ = sidx // (N_CORES * CK)
    r_ = (sidx % (N_CORES * CK)) // CK
    sl = sidx % CK
    perm = r_ * S_LOC + c_ * CK + sl
    bias_perm = bias_orig[perm]                                   # (S,)
    mb = np.ascontiguousarray(bias_perm.reshape(N_ST, 128).T)     # (128, N_ST)

    in_maps = []
    for c in range(N_CORES):
        in_maps.append({
            "xT": np.ascontiguousarray(xT[:, c * S_LOC:(c + 1) * S_LOC]),
            "wkvT": wkvT,
            "qT": np.ascontiguousarray(qTf[:, c * QR:(c + 1) * QR]),
            "qres": np.ascontiguousarray(flat[c * QR:(c + 1) * QR, :]),
            "woutT": woutT,
            "maskb": mb,
            "gamma": gamma,
            "beta": beta,
        })
    return in_maps


def kernel(**inputs) -> np.ndarray:
    global _NC_CACHE
    in_maps = _make_in_maps(inputs)
    if _NC_CACHE is None:
        _NC_CACHE = build_program()
    res = run_bass_kernel_spmd(_NC_CACHE, in_maps, list(range(N_CORES)))
    out = np.concatenate([res.results[c]["y"] for c in range(N_CORES)], axis=0)
    return np.ascontiguousarray(out.reshape(J, Q, D).astype(np.float32))


# revision 14
# speedup vs baseline: 1.1171x; 1.1171x over previous
"""ChunkCrossAttention Trainium2 kernel (v3: chunked AllGather-KV pipeline).

Math (per reference):
  x = chunk_embeddings[0]                      # (S, L)
  k, v = split(x @ W_kv.T)                     # (S, D) each
  scores = einsum('jqd,sd->jqs', q, k) / sqrt(D), masked
  attn = softmax(scores, -1)
  out = (attn @ v) @ W_out.T + q  -> LayerNorm(gamma, beta)

Strategy (8 NeuronCores):
  - KV projection sharded over S: each core projects its own 512 keys
    in 4 chunks of 128, fp8 DoubleRow matmuls (wkvT resident in SBUF,
    xT streamed). After each chunk the k^T/v' fp8 payload is
    AllGathered; the 4 AGs pipeline behind each other and behind the
    projection, so foreign K/V arrives while the PE is still busy.
  - W_out folded into v (v' = v @ W_out.T scaled, plus a ones column
    that makes the attention matmul emit the softmax denominator).
  - Attention is query-sharded: each core handles its own 1024 query
    rows over all 4096 keys (exp without max-subtraction, shifted -3
    to stay in fp8e4 range; mask folded into the Exp bias). Scores run
    fp8 DoubleRow; attn@v' accumulates over the 32 key tiles in PSUM.
    The attn@v' matmuls for key-tile-pair n are issued after the score
    matmuls of pair n+1, so the PE never stalls behind the Exp
    activations (software pipelining).
  - Epilogue (denominator divide, residual, LayerNorm) per 128-row
    q tile; rsqrt computed on the vector engine (fast-inverse-sqrt +
    2 Newton steps) so the scalar engine keeps its Exp table.
  - fp8 scaling: W_kv pre-scaled x64 host-side (its std 1/64 would
    land in fp8e4's subnormal range), divided back out during the
    PSUM->fp8 copies; W_out likewise x16.
"""
import sys

sys.path.insert(0, "/opt/trn_rl_repo")

import numpy as np

import concourse.bacc as bacc
import concourse.mybir as mybir
import concourse.tile as tile
from concourse.bass_utils import run_bass_kernel_spmd

N_CORES = 8
J, Q, D = 64, 128, 256
S, L = 4096, 4096
S_LOC = S // N_CORES          # 512 keys per core
QR = (J // N_CORES) * Q       # 1024 query rows per core
CH = 2                        # phase-1 key chunks per core
CK = S_LOC // CH              # 256 keys per chunk
WARM_FILL = 220               # dummy matmuls bridging the AllGather wait
AT_LAG = 2                    # attn@v' matmuls trail the score matmuls
DP = 272                      # at free dim: 256 outputs + denom + pad to x16
N_ST = S // 128               # 32 key tiles
LN_EPS = 1e-5
SCALE = 1.0 / np.sqrt(D)
EXP_SHIFT = -3.0              # folded into maskb host-side
KV_SCALE = 64.0               # host premultiplies wkvT
WO_SCALE = 16.0               # host premultiplies woutT
AT_DR = False                 # DoubleRow for attn@v' matmuls

F32 = mybir.dt.float32
I32 = mybir.dt.int32
FP8 = mybir.dt.float8e4
AF = mybir.ActivationFunctionType
ALU = mybir.AluOpType
DR = mybir.MatmulPerfMode.DoubleRow

RSQRT_MAGIC = 0x5f3759df


def build_program():
    nc = bacc.Bacc(None, num_devices=N_CORES)

    xT = nc.declare_dram_parameter("xT", [L, S_LOC], FP8, isOutput=False)
    wkvT = nc.declare_dram_parameter("wkvT", [L, 2 * D], FP8, isOutput=False)
    qT = nc.declare_dram_parameter("qT", [D, QR], FP8, isOutput=False)
    qres = nc.declare_dram_parameter("qres", [QR, D], F32, isOutput=False)
    woutT = nc.declare_dram_parameter("woutT", [D, D], FP8, isOutput=False)
    maskb = nc.declare_dram_parameter("maskb", [128, N_ST], F32, isOutput=False)
    gamma = nc.declare_dram_parameter("gamma", [D], F32, isOutput=False)
    beta = nc.declare_dram_parameter("beta", [D], F32, isOutput=False)
    y = nc.declare_dram_parameter("y", [QR, D], F32, isOutput=True)

    # per-chunk AllGather payload: kT (2*CK) + vp (CK/128 * DP) fp8/partition
    NSS = CK // 128
    AGW = 2 * CK + NSS * DP
    ag_in = [nc.dram_tensor(f"ag_in{c}", [128, AGW], FP8) for c in range(CH)]
    ag_out = [nc.dram_tensor(f"ag_out{c}", [N_CORES, 128, AGW], FP8,
                             addr_space="Shared") for c in range(CH)]

    import concourse.bass as bass

    with tile.TileContext(nc) as tc:
        with tc.tile_pool(name="singles", bufs=1) as singles, \
             tc.tile_pool(name="xw", bufs=3) as xw, \
             tc.tile_pool(name="kv", bufs=1) as kvp, \
             tc.tile_pool(name="exp", bufs=4) as epool, \
             tc.tile_pool(name="small", bufs=8) as small:

            # ---- resident inputs (wkv split in 4 so matmuls start early) ----
            wkv_sb = singles.tile([128, L // 128, 2 * D], FP8)
            for w4 in range(4):
                nc.sync.dma_start(
                    out=wkv_sb[:, w4 * 8:(w4 + 1) * 8, :],
                    in_=wkvT[w4 * 1024:(w4 + 1) * 1024, :].rearrange(
                        "(a p) e -> p a e", p=128))
            qT_sb = singles.tile([128, 2, QR], FP8)
            nc.gpsimd.dma_start(
                out=qT_sb, in_=qT.rearrange("(dc p) q -> p dc q", p=128))
            woutT_sb = singles.tile([128, 2, D], FP8)
            nc.gpsimd.dma_start(
                out=woutT_sb, in_=woutT.rearrange("(dc p) e -> p dc e", p=128))
            maskb_sb = singles.tile([128, N_ST], F32)
            nc.gpsimd.dma_start(out=maskb_sb, in_=maskb[:, :])
            g_ap = gamma[:]
            gamma_sb = singles.tile([128, D], F32)
            nc.gpsimd.dma_start(out=gamma_sb, in_=bass.AP(
                tensor=g_ap.tensor, offset=g_ap.offset,
                ap=[[0, 128], g_ap.ap[0]]))
            b_ap = beta[:]
            beta_sb = singles.tile([128, D], F32)
            nc.gpsimd.dma_start(out=beta_sb, in_=bass.AP(
                tensor=b_ap.tensor, offset=b_ap.offset,
                ap=[[0, 128], b_ap.ap[0]]))

            kT_all = singles.tile([128, 2, S], FP8)
            vp_all = singles.tile([128, N_ST, DP], FP8)

            # ---- phase 1: project local keys, chunk by chunk; AG each ----
            ps1 = tc.tile_pool(name="ps_kv", bufs=1, space="PSUM")
            ps_kv = ps1.__enter__()
            for c in range(CH):
                acc = [ps_kv.tile([128, 512], F32, tag=f"acc{h}",
                                  name=f"acc{c}_{h}") for h in range(4)]
                for lb in range(L // 512):
                    xt = xw.tile([128, 4, CK], FP8, tag="xt")
                    nc.sync.dma_start(
                        out=xt,
                        in_=xT[lb * 512:(lb + 1) * 512,
                               c * CK:(c + 1) * CK].rearrange(
                            "(a p) s -> p a s", p=128))
                    for ap_ in range(2):
                        A = lb * 4 + 2 * ap_
                        first = lb == 0 and ap_ == 0
                        last = lb == L // 512 - 1 and ap_ == 1
                        for h in range(4):
                            nc.tensor.matmul(
                                acc[h][:, 0:CK],
                                wkv_sb[:, A:A + 2, h * 128:(h + 1) * 128],
                                xt[:, 2 * ap_:2 * ap_ + 2, :],
                                start=first, stop=last, perf_mode=DR)

                # quantize k^T, v^T to fp8 (undo the x64 W_kv prescale)
                kt_c = kvp.tile([128, 2, CK], FP8, name=f"ktc{c}")
                vt_c = kvp.tile([128, 2, CK], FP8, name=f"vtc{c}")
                for dc in range(2):
                    nc.scalar.activation(out=kt_c[:, dc, :], in_=acc[dc][:, 0:CK],
                                         func=AF.Copy, scale=1.0 / KV_SCALE)
                    nc.scalar.activation(out=vt_c[:, dc, :],
                                         in_=acc[2 + dc][:, 0:CK],
                                         func=AF.Copy, scale=1.0 / KV_SCALE)
                # v' = v @ W_out.T (scaled), plus ones column at DP col 256
                vp_c = kvp.tile([128, NSS, DP], FP8, name=f"vpc{c}")
                nc.vector.memset(vp_c, 0.0)
                for ss in range(NSS):
                    pv = ps_kv.tile([128, 512], F32, tag="pv",
                                    name=f"pv{c}_{ss}")
                    nc.tensor.matmul(pv[:, 0:D],
                                     vt_c[:, :, ss * 128:(ss + 1) * 128],
                                     woutT_sb, start=True, stop=True,
                                     perf_mode=DR)
                    nc.scalar.activation(out=vp_c[:, ss, 0:D], in_=pv[:, 0:D],
                                         func=AF.Copy, scale=1.0 / WO_SCALE)
                nc.vector.memset(vp_c[:, :, D:D + 1], 1.0)

                # ship local chunk, gather everyone's
                nc.sync.dma_start(
                    out=ag_in[c][:, 0:2 * CK].rearrange(
                        "p (dc s) -> p dc s", dc=2),
                    in_=kt_c)
                nc.sync.dma_start(
                    out=ag_in[c][:, 2 * CK:AGW].rearrange(
                        "p (i f) -> p i f", i=NSS),
                    in_=vp_c)
                nc.gpsimd.collective_compute(
                    "AllGather", ALU.bypass,
                    replica_groups=[list(range(N_CORES))],
                    ins=[ag_in[c][:, :]], outs=[ag_out[c][:, :, :]])
                for r in range(N_CORES):
                    slot = c * N_CORES + r
                    nc.sync.dma_start(
                        out=kT_all[:, :, slot * CK:(slot + 1) * CK],
                        in_=ag_out[c][r, :, 0:2 * CK].rearrange(
                            "p (dc s) -> p dc s", dc=2))
                    nc.sync.dma_start(
                        out=vp_all[:, slot * NSS:(slot + 1) * NSS, :],
                        in_=ag_out[c][r, :, 2 * CK:AGW].rearrange(
                            "p (i f) -> p i f", i=NSS))

            # qres is only needed by the epilogue; load it after the
            # phase-1 x/w streams so it doesn't starve them
            qres_sb = singles.tile([128, QR // 128, D], F32)
            nc.gpsimd.dma_start(
                out=qres_sb, in_=qres.rearrange("(t p) d -> p t d", p=128))

            # dummy matmuls bridge the AllGather wait so the PE's HAM
            # clock gate stays at full rate into the attention phase
            scr = small.tile([128, 128], FP8, tag="scr")
            nc.vector.memset(scr, 0.0)
            warm_ps = ps_kv.tile([128, 512], F32, tag="warm")
            for _ in range(WARM_FILL):
                nc.tensor.matmul(warm_ps[:, 0:128], scr, scr,
                                 start=True, stop=True)
            ps1.__exit__(None, None, None)

            # ---- phase 2: q-sharded attention over all keys ----
            ps2 = tc.tile_pool(name="ps_at", bufs=1, space="PSUM")
            ps_at = ps2.__enter__()
            ps3 = tc.tile_pool(name="ps_sc", bufs=4, space="PSUM")
            ps_sc = ps3.__enter__()

            NP = N_ST // 2        # 16 key-tile pairs

            def emit_at(at, stp, ex2):
                for qt in range(4):
                    if AT_DR:
                        nc.tensor.matmul(
                            at[qt][:, 0:DP],
                            ex2[:, :, qt * 128:(qt + 1) * 128],
                            vp_all[:, stp * 2:stp * 2 + 2, :],
                            start=(stp == 0), stop=(stp == NP - 1),
                            perf_mode=DR)
                    else:
                        for par in range(2):
                            nc.tensor.matmul(
                                at[qt][:, 0:DP],
                                ex2[:, par, qt * 128:(qt + 1) * 128],
                                vp_all[:, stp * 2 + par, :],
                                start=(stp == 0 and par == 0),
                                stop=(stp == NP - 1 and par == 1))

            y_r = y.rearrange("(hh t p) d -> hh t p d", hh=2, t=4)
            for half in range(2):
                at = [ps_at.tile([128, 512], F32, tag=f"at{qt}",
                                 name=f"at{half}_{qt}") for qt in range(4)]
                pend = []
                for stp in range(NP):
                    ex2 = epool.tile([128, 2, 512], FP8, tag="ex")
                    for par in range(2):
                        st = stp * 2 + par
                        sc = ps_sc.tile([128, 512], F32, tag="sc")
                        nc.tensor.matmul(
                            sc, kT_all[:, :, st * 128:(st + 1) * 128],
                            qT_sb[:, :, half * 512:(half + 1) * 512],
                            start=True, stop=True, perf_mode=DR)
                        nc.scalar.activation(out=ex2[:, par, :], in_=sc,
                                             func=AF.Exp,
                                             bias=maskb_sb[:, st:st + 1],
                                             scale=SCALE)
                    pend.append((stp, ex2))
                    if len(pend) > AT_LAG:
                        pstp, pex = pend.pop(0)
                        emit_at(at, pstp, pex)
                for pstp, pex in pend:
                    emit_at(at, pstp, pex)

                # ---- epilogue: denom divide, residual, LayerNorm ----
                h_half = singles.tile([128, 4, D], F32, name=f"h_half{half}")
                vars4 = small.tile([128, 4], F32, tag="vars4")
                mus4 = small.tile([128, 4], F32, tag="mus4")
                for qt in range(4):
                    t = 4 * half + qt
                    hs = h_half[:, qt, :]
                    rec = small.tile([128, 1], F32, tag="rec")
                    nc.vector.reciprocal(out=rec, in_=at[qt][:, D:D + 1])
                    nc.vector.tensor_scalar_mul(out=hs, in0=at[qt][:, 0:D],
                                                scalar1=rec)
                    nc.vector.tensor_add(out=hs, in0=hs, in1=qres_sb[:, t, :])
                    stats = small.tile([128, 6], F32, tag="stats")
                    nc.vector.bn_stats(out=stats, in_=hs)
                    mv = small.tile([128, 2], F32, tag="mv")
                    nc.vector.bn_aggr(out=mv, in_=stats)
                    nc.vector.tensor_copy(out=mus4[:, qt:qt + 1], in_=mv[:, 0:1])
                    nc.vector.tensor_copy(out=vars4[:, qt:qt + 1], in_=mv[:, 1:2])
                # rstd = 1/sqrt(var+eps) on DVE: fast-inv-sqrt + 2 Newton steps
                veps = small.tile([128, 4], F32, tag="veps")
                nc.vector.tensor_scalar(out=veps, in0=vars4, scalar1=1.0,
                                        scalar2=LN_EPS, op0=ALU.mult,
                                        op1=ALU.add)
                yv = small.tile([128, 4], F32, tag="yv")
                yv_i = yv.bitcast(I32)
                nc.vector.tensor_scalar(out=yv_i, in0=veps.bitcast(I32),
                                        scalar1=1, scalar2=None,
                                        op0=ALU.logical_shift_right)
                nc.vector.tensor_scalar(out=yv_i, in0=yv_i,
                                        scalar1=RSQRT_MAGIC, scalar2=-1,
                                        op0=ALU.subtract, op1=ALU.mult)
                tn = small.tile([128, 4], F32, tag="tn")
                for _ in range(2):
                    nc.vector.tensor_mul(out=tn, in0=yv, in1=yv)
                    nc.vector.tensor_mul(out=tn, in0=tn, in1=veps)
                    nc.vector.tensor_scalar(out=tn, in0=tn, scalar1=-0.5,
                                            scalar2=1.5, op0=ALU.mult,
                                            op1=ALU.add)
                    nc.vector.tensor_mul(out=yv, in0=yv, in1=tn)
                for qt in range(4):
                    hs = h_half[:, qt, :]
                    nc.vector.tensor_scalar(out=hs, in0=hs,
                                            scalar1=mus4[:, qt:qt + 1],
                                            scalar2=yv[:, qt:qt + 1],
                                            op0=ALU.subtract, op1=ALU.mult)
                    nc.vector.tensor_mul(out=hs, in0=hs, in1=gamma_sb)
                    nc.vector.tensor_add(out=hs, in0=hs, in1=beta_sb)
                    nc.gpsimd.dma_start(out=y_r[half, qt], in_=hs)

            ps3.__exit__(None, None, None)
            ps2.__exit__(None, None, None)

    nc.finalize()
    return nc


_NC_CACHE = None


def _make_in_maps(inputs):
    jq = np.asarray(inputs["justice_queries"], dtype=np.float32)
    x = np.asarray(inputs["chunk_embeddings"], dtype=np.float32)[0]
    mask = np.asarray(inputs["chunk_mask"])
    wkv = np.asarray(inputs["W_kv"], dtype=np.float32)
    wout = np.asarray(inputs["W_out"], dtype=np.float32)
    gamma = np.asarray(inputs["ln_gamma"], dtype=np.float32)
    beta = np.asarray(inputs["ln_beta"], dtype=np.float32)

    import ml_dtypes
    fp8 = ml_dtypes.float8_e4m3
    xT = np.ascontiguousarray(x.T.astype(fp8))                    # (L, S)
    wkvT = np.ascontiguousarray((wkv.T * KV_SCALE).astype(fp8))   # (L, 2D)
    flat = np.ascontiguousarray(jq.reshape(J * Q, D))             # (8192, D)
    qTf = flat.T.astype(fp8)                                      # (D, 8192)
    woutT = np.ascontiguousarray((wout.T * WO_SCALE).astype(fp8))  # (D, D)

    # mask bias in the AllGather key order: s = c*(8*CK) + r*CK + sloc
    # maps to original key r*S_LOC + c*CK + sloc; -3 shift keeps exp in
    # fp8e4 range (ratios cancel via the denominator).
    bias_orig = np.where(mask != 0, 0.0, -1e30).astype(np.float32) + EXP_SHIFT
    sidx = np.arange(S)
    c_ = sidx // (N_CORES * CK)
    r_ = (sidx % (N_CORES * CK)) // CK
    sl = sidx % CK
    perm = r_ * S_LOC + c_ * CK + sl
    bias_perm = bias_orig[perm]                                   # (S,)
    mb = np.ascontiguousarray(bias_perm.reshape(N_ST, 128).T)     # (128, N_ST)

    in_maps = []
    for c in range(N_CORES):
        in_maps.append({
            "xT": np.ascontiguousarray(xT[:, c * S_LOC:(c + 1) * S_LOC]),
            "wkvT": wkvT,
            "qT": np.ascontiguousarray(qTf[:, c * QR:(c + 1) * QR]),
            "qres": np.ascontiguousarray(flat[c * QR:(c + 1) * QR, :]),
            "woutT": woutT,
            "maskb": mb,
            "gamma": gamma,
            "beta": beta,
        })
    return in_maps


def kernel(**inputs) -> np.ndarray:
    global _NC_CACHE
    in_maps = _make_in_maps(inputs)
    if _NC_CACHE is None:
        _NC_CACHE = build_program()
    res = run_bass_kernel_spmd(_NC_CACHE, in_maps, list(range(N_CORES)))
    out = np.concatenate([res.results[c]["y"] for c in range(N_CORES)], axis=0)
    return np.ascontiguousarray(out.reshape(J, Q, D).astype(np.float32))


# revision 24
# speedup vs baseline: 1.2374x; 1.1077x over previous
"""ChunkCrossAttention Trainium2 kernel (v3: chunked AllGather-KV pipeline).

Math (per reference):
  x = chunk_embeddings[0]                      # (S, L)
  k, v = split(x @ W_kv.T)                     # (S, D) each
  scores = einsum('jqd,sd->jqs', q, k) / sqrt(D), masked
  attn = softmax(scores, -1)
  out = (attn @ v) @ W_out.T + q  -> LayerNorm(gamma, beta)

Strategy (8 NeuronCores):
  - KV projection sharded over S: each core projects its own 512 keys
    in 4 chunks of 128, fp8 DoubleRow matmuls (wkvT resident in SBUF,
    xT streamed). After each chunk the k^T/v' fp8 payload is
    AllGathered; the 4 AGs pipeline behind each other and behind the
    projection, so foreign K/V arrives while the PE is still busy.
  - W_out folded into v (v' = v @ W_out.T scaled, plus a ones column
    that makes the attention matmul emit the softmax denominator).
  - Attention is query-sharded: each core handles its own 1024 query
    rows over all 4096 keys (exp without max-subtraction, shifted -3
    to stay in fp8e4 range; mask folded into the Exp bias). Scores run
    fp8 DoubleRow; attn@v' accumulates over the 32 key tiles in PSUM.
    The attn@v' matmuls for key-tile-pair n are issued after the score
    matmuls of pair n+1, so the PE never stalls behind the Exp
    activations (software pipelining).
  - Epilogue (denominator divide, residual, LayerNorm) per 128-row
    q tile; rsqrt computed on the vector engine (fast-inverse-sqrt +
    2 Newton steps) so the scalar engine keeps its Exp table.
  - fp8 scaling: W_kv pre-scaled x64 host-side (its std 1/64 would
    land in fp8e4's subnormal range), divided back out during the
    PSUM->fp8 copies; W_out likewise x16.
"""
import sys

sys.path.insert(0, "/opt/trn_rl_repo")

import numpy as np

import concourse.bacc as bacc
import concourse.mybir as mybir
import concourse.tile as tile
from concourse.bass_utils import run_bass_kernel_spmd

N_CORES = 8
J, Q, D = 64, 128, 256
S, L = 4096, 4096
S_LOC = S // N_CORES          # 512 keys per core
QR = (J // N_CORES) * Q       # 1024 query rows per core
CH = 2                        # phase-1 key chunks per core
CK = S_LOC // CH              # 256 keys per chunk
AT_LAG = 2                    # attn@v' matmuls trail the score matmuls
DP = 272                      # at free dim: 256 outputs + denom + pad to x16
N_ST = S // 128               # 32 key tiles
LN_EPS = 1e-5
SCALE = 1.0 / np.sqrt(D)
EXP_SHIFT = -3.0              # folded into maskb host-side
KV_SCALE = 64.0               # host premultiplies wkvT
WO_SCALE = 16.0               # host premultiplies woutT
AT_DR = True                  # DoubleRow for attn@v' matmuls

F32 = mybir.dt.float32
I32 = mybir.dt.int32
FP8 = mybir.dt.float8e4
AF = mybir.ActivationFunctionType
ALU = mybir.AluOpType
DR = mybir.MatmulPerfMode.DoubleRow

RSQRT_MAGIC = 0x5f3759df


def build_program():
    nc = bacc.Bacc(None, num_devices=N_CORES)

    # xT/wkvT come host-prepermuted to partition-major layouts so the
    # phase-1 DMAs are long contiguous runs (1-4KB) instead of 256-512B
    xTh = nc.declare_dram_parameter("xTh", [128, CH, L // 128, CK], FP8,
                                    isOutput=False)
    wkvTh = nc.declare_dram_parameter("wkvTh", [128, L // 128, 2 * D], FP8,
                                      isOutput=False)
    qT = nc.declare_dram_parameter("qT", [D, QR], FP8, isOutput=False)
    qres = nc.declare_dram_parameter("qres", [QR, D], F32, isOutput=False)
    woutT = nc.declare_dram_parameter("woutT", [D, D], FP8, isOutput=False)
    maskb = nc.declare_dram_parameter("maskb", [128, N_ST], F32, isOutput=False)
    gamma = nc.declare_dram_parameter("gamma", [D], F32, isOutput=False)
    beta = nc.declare_dram_parameter("beta", [D], F32, isOutput=False)
    y = nc.declare_dram_parameter("y", [QR, D], F32, isOutput=True)

    # per-chunk AllGather payload: kT (2*CK) + vp (CK/128 * DP) fp8/partition
    NSS = CK // 128
    AGW = 2 * CK + NSS * DP
    ag_in = [nc.dram_tensor(f"ag_in{c}", [128, AGW], FP8) for c in range(CH)]
    ag_out = [nc.dram_tensor(f"ag_out{c}", [N_CORES, 128, AGW], FP8,
                             addr_space="Shared") for c in range(CH)]

    import concourse.bass as bass

    with tile.TileContext(nc) as tc:
        with tc.tile_pool(name="singles", bufs=1) as singles, \
             tc.tile_pool(name="xw", bufs=3) as xw, \
             tc.tile_pool(name="kv", bufs=1) as kvp, \
             tc.tile_pool(name="exp", bufs=4) as epool, \
             tc.tile_pool(name="small", bufs=8) as small:

            # ---- resident inputs ----
            # wkv is issued interleaved with the first chunk's x stream below
            wkv_sb = singles.tile([128, L // 128, 2 * D], FP8)
            qT_sb = singles.tile([128, 2, QR], FP8)
            nc.gpsimd.dma_start(
                out=qT_sb, in_=qT.rearrange("(dc p) q -> p dc q", p=128))
            woutT_sb = singles.tile([128, 2, D], FP8)
            nc.gpsimd.dma_start(
                out=woutT_sb, in_=woutT.rearrange("(dc p) e -> p dc e", p=128))
            maskb_sb = singles.tile([128, N_ST], F32)
            nc.gpsimd.dma_start(out=maskb_sb, in_=maskb[:, :])
            g_ap = gamma[:]
            gamma_sb = singles.tile([128, D], F32)
            nc.gpsimd.dma_start(out=gamma_sb, in_=bass.AP(
                tensor=g_ap.tensor, offset=g_ap.offset,
                ap=[[0, 128], g_ap.ap[0]]))
            b_ap = beta[:]
            beta_sb = singles.tile([128, D], F32)
            nc.gpsimd.dma_start(out=beta_sb, in_=bass.AP(
                tensor=b_ap.tensor, offset=b_ap.offset,
                ap=[[0, 128], b_ap.ap[0]]))

            kT_all = singles.tile([128, 2, S], FP8)
            vp_all = singles.tile([128, N_ST, DP], FP8)

            # ---- phase 1: project local keys, chunk by chunk; AG each ----
            ps1 = tc.tile_pool(name="ps_kv", bufs=1, space="PSUM")
            ps_kv = ps1.__enter__()
            for c in range(CH):
                acc = [ps_kv.tile([128, 512], F32, tag=f"acc{h}",
                                  name=f"acc{c}_{h}") for h in range(4)]
                for lb in range(L // 512):
                    if c == 0 and lb % 2 == 0:
                        w4 = lb // 2
                        nc.sync.dma_start(
                            out=wkv_sb[:, w4 * 8:(w4 + 1) * 8, :],
                            in_=wkvTh[:, w4 * 8:(w4 + 1) * 8, :])
                    xt = xw.tile([128, 4, CK], FP8, tag="xt")
                    nc.sync.dma_start(
                        out=xt, in_=xTh[:, c, lb * 4:(lb + 1) * 4, :])
                    for ap_ in range(2):
                        A = lb * 4 + 2 * ap_
                        first = lb == 0 and ap_ == 0
                        last = lb == L // 512 - 1 and ap_ == 1
                        for h in range(4):
                            nc.tensor.matmul(
                                acc[h][:, 0:CK],
                                wkv_sb[:, A:A + 2, h * 128:(h + 1) * 128],
                                xt[:, 2 * ap_:2 * ap_ + 2, :],
                                start=first, stop=last, perf_mode=DR)

                # quantize k^T, v^T to fp8 (undo the x64 W_kv prescale);
                # split between vector+scalar engines to shorten the chain
                # to the AllGather trigger
                kt_c = kvp.tile([128, 2, CK], FP8, name=f"ktc{c}")
                vt_c = kvp.tile([128, 2, CK], FP8, name=f"vtc{c}")
                for dc in range(2):
                    nc.vector.tensor_scalar_mul(out=kt_c[:, dc, :],
                                                in0=acc[dc][:, 0:CK],
                                                scalar1=1.0 / KV_SCALE)
                    nc.scalar.activation(out=vt_c[:, dc, :],
                                         in_=acc[2 + dc][:, 0:CK],
                                         func=AF.Copy, scale=1.0 / KV_SCALE)
                # v' = v @ W_out.T (scaled), plus ones column at DP col 256
                vp_c = kvp.tile([128, NSS, DP], FP8, name=f"vpc{c}")
                nc.vector.memset(vp_c, 0.0)
                for ss in range(NSS):
                    pv = ps_kv.tile([128, 512], F32, tag="pv",
                                    name=f"pv{c}_{ss}")
                    nc.tensor.matmul(pv[:, 0:D],
                                     vt_c[:, :, ss * 128:(ss + 1) * 128],
                                     woutT_sb, start=True, stop=True,
                                     perf_mode=DR)
                    nc.vector.tensor_scalar_mul(out=vp_c[:, ss, 0:D],
                                                in0=pv[:, 0:D],
                                                scalar1=1.0 / WO_SCALE)
                nc.vector.memset(vp_c[:, :, D:D + 1], 1.0)

                # ship local chunk, gather everyone's
                nc.sync.dma_start(
                    out=ag_in[c][:, 0:2 * CK].rearrange(
                        "p (dc s) -> p dc s", dc=2),
                    in_=kt_c)
                nc.sync.dma_start(
                    out=ag_in[c][:, 2 * CK:AGW].rearrange(
                        "p (i f) -> p i f", i=NSS),
                    in_=vp_c)
                nc.gpsimd.collective_compute(
                    "AllGather", ALU.bypass,
                    replica_groups=[list(range(N_CORES))],
                    ins=[ag_in[c][:, :]], outs=[ag_out[c][:, :, :]])
                for r in range(N_CORES):
                    slot = c * N_CORES + r
                    nc.sync.dma_start(
                        out=kT_all[:, :, slot * CK:(slot + 1) * CK],
                        in_=ag_out[c][r, :, 0:2 * CK].rearrange(
                            "p (dc s) -> p dc s", dc=2))
                    nc.sync.dma_start(
                        out=vp_all[:, slot * NSS:(slot + 1) * NSS, :],
                        in_=ag_out[c][r, :, 2 * CK:AGW].rearrange(
                            "p (i f) -> p i f", i=NSS))

            # qres is only needed by the epilogue; load it after the
            # phase-1 x/w streams so it doesn't starve them
            qres_sb = singles.tile([128, QR // 128, D], F32)
            nc.gpsimd.dma_start(
                out=qres_sb, in_=qres.rearrange("(t p) d -> p t d", p=128))
            ps1.__exit__(None, None, None)

            # ---- phase 2: q-sharded attention over all keys ----
            ps2 = tc.tile_pool(name="ps_at", bufs=1, space="PSUM")
            ps_at = ps2.__enter__()
            ps3 = tc.tile_pool(name="ps_sc", bufs=4, space="PSUM")
            ps_sc = ps3.__enter__()

            NP = N_ST // 2        # 16 key-tile pairs

            def emit_at(at, stp, ex2):
                for qt in range(4):
                    if AT_DR:
                        nc.tensor.matmul(
                            at[qt][:, 0:DP],
                            ex2[:, :, qt * 128:(qt + 1) * 128],
                            vp_all[:, stp * 2:stp * 2 + 2, :],
                            start=(stp == 0), stop=(stp == NP - 1),
                            perf_mode=DR)
                    else:
                        for par in range(2):
                            nc.tensor.matmul(
                                at[qt][:, 0:DP],
                                ex2[:, par, qt * 128:(qt + 1) * 128],
                                vp_all[:, stp * 2 + par, :],
                                start=(stp == 0 and par == 0),
                                stop=(stp == NP - 1 and par == 1))

            y_r = y.rearrange("(hh t p) d -> hh t p d", hh=2, t=4)
            for half in range(2):
                at = [ps_at.tile([128, 512], F32, tag=f"at{qt}",
                                 name=f"at{half}_{qt}") for qt in range(4)]
                pend = []
                for stp in range(NP):
                    ex2 = epool.tile([128, 2, 512], FP8, tag="ex")
                    for par in range(2):
                        st = stp * 2 + par
                        sc = ps_sc.tile([128, 512], F32, tag="sc")
                        nc.tensor.matmul(
                            sc, kT_all[:, :, st * 128:(st + 1) * 128],
                            qT_sb[:, :, half * 512:(half + 1) * 512],
                            start=True, stop=True, perf_mode=DR)
                        nc.scalar.activation(out=ex2[:, par, :], in_=sc,
                                             func=AF.Exp,
                                             bias=maskb_sb[:, st:st + 1],
                                             scale=SCALE)
                    pend.append((stp, ex2))
                    if len(pend) > AT_LAG:
                        pstp, pex = pend.pop(0)
                        emit_at(at, pstp, pex)
                for pstp, pex in pend:
                    emit_at(at, pstp, pex)

                # ---- epilogue: denom divide, residual, LayerNorm ----
                h_half = singles.tile([128, 4, D], F32, name=f"h_half{half}")
                vars4 = small.tile([128, 4], F32, tag="vars4")
                mus4 = small.tile([128, 4], F32, tag="mus4")
                for qt in range(4):
                    t = 4 * half + qt
                    hs = h_half[:, qt, :]
                    # LayerNorm is invariant to positive per-row scaling, so
                    # normalize num + denom*qres instead of num/denom + qres
                    nc.vector.tensor_scalar_mul(out=hs, in0=qres_sb[:, t, :],
                                                scalar1=at[qt][:, D:D + 1])
                    nc.vector.tensor_add(out=hs, in0=hs, in1=at[qt][:, 0:D])
                    stats = small.tile([128, 6], F32, tag="stats")
                    nc.vector.bn_stats(out=stats, in_=hs)
                    mv = small.tile([128, 2], F32, tag="mv")
                    nc.vector.bn_aggr(out=mv, in_=stats)
                    nc.vector.tensor_copy(out=mus4[:, qt:qt + 1], in_=mv[:, 0:1])
                    nc.vector.tensor_copy(out=vars4[:, qt:qt + 1], in_=mv[:, 1:2])
                # rstd = 1/sqrt(var+eps) on DVE: fast-inv-sqrt + 2 Newton steps
                veps = small.tile([128, 4], F32, tag="veps")
                nc.vector.tensor_scalar(out=veps, in0=vars4, scalar1=1.0,
                                        scalar2=LN_EPS, op0=ALU.mult,
                                        op1=ALU.add)
                yv = small.tile([128, 4], F32, tag="yv")
                yv_i = yv.bitcast(I32)
                nc.vector.tensor_scalar(out=yv_i, in0=veps.bitcast(I32),
                                        scalar1=1, scalar2=None,
                                        op0=ALU.logical_shift_right)
                nc.vector.tensor_scalar(out=yv_i, in0=yv_i,
                                        scalar1=RSQRT_MAGIC, scalar2=-1,
                                        op0=ALU.subtract, op1=ALU.mult)
                tn = small.tile([128, 4], F32, tag="tn")
                for _ in range(2):
                    nc.vector.tensor_mul(out=tn, in0=yv, in1=yv)
                    nc.vector.tensor_mul(out=tn, in0=tn, in1=veps)
                    nc.vector.tensor_scalar(out=tn, in0=tn, scalar1=-0.5,
                                            scalar2=1.5, op0=ALU.mult,
                                            op1=ALU.add)
                    nc.vector.tensor_mul(out=yv, in0=yv, in1=tn)
                for qt in range(4):
                    hs = h_half[:, qt, :]
                    nc.vector.tensor_scalar(out=hs, in0=hs,
                                            scalar1=mus4[:, qt:qt + 1],
                                            scalar2=yv[:, qt:qt + 1],
                                            op0=ALU.subtract, op1=ALU.mult)
                    nc.vector.tensor_mul(out=hs, in0=hs, in1=gamma_sb)
                    nc.vector.tensor_add(out=hs, in0=hs, in1=beta_sb)
                    nc.gpsimd.dma_start(out=y_r[half, qt], in_=hs)

            ps3.__exit__(None, None, None)
            ps2.__exit__(None, None, None)

    nc.finalize()
    return nc


_NC_CACHE = None


def _make_in_maps(inputs):
    jq = np.asarray(inputs["justice_queries"], dtype=np.float32)
    x = np.asarray(inputs["chunk_embeddings"], dtype=np.float32)[0]
    mask = np.asarray(inputs["chunk_mask"])
    wkv = np.asarray(inputs["W_kv"], dtype=np.float32)
    wout = np.asarray(inputs["W_out"], dtype=np.float32)
    gamma = np.asarray(inputs["ln_gamma"], dtype=np.float32)
    beta = np.asarray(inputs["ln_beta"], dtype=np.float32)

    import ml_dtypes
    fp8 = ml_dtypes.float8_e4m3
    xT = np.ascontiguousarray(x.T.astype(fp8))                    # (L, S)
    # partition-major permutations for contiguous phase-1 DMA
    wkvT = (wkv.T * KV_SCALE).astype(fp8)                         # (L, 2D)
    wkvTh = np.ascontiguousarray(
        wkvT.reshape(L // 128, 128, 2 * D).transpose(1, 0, 2))   # (128,32,2D)
    flat = np.ascontiguousarray(jq.reshape(J * Q, D))             # (8192, D)
    qTf = flat.T.astype(fp8)                                      # (D, 8192)
    woutT = np.ascontiguousarray((wout.T * WO_SCALE).astype(fp8))  # (D, D)

    # mask bias in the AllGather key order: s = c*(8*CK) + r*CK + sloc
    # maps to original key r*S_LOC + c*CK + sloc; -3 shift keeps exp in
    # fp8e4 range (ratios cancel via the denominator).
    bias_orig = np.where(mask != 0, 0.0, -1e30).astype(np.float32) + EXP_SHIFT
    sidx = np.arange(S)
    c_ = sidx // (N_CORES * CK)
    r_ = (sidx % (N_CORES * CK)) // CK
    sl = sidx % CK
    perm = r_ * S_LOC + c_ * CK + sl
    bias_perm = bias_orig[perm]                                   # (S,)
    mb = np.ascontiguousarray(bias_perm.reshape(N_ST, 128).T)     # (128, N_ST)

    in_maps = []
    for c in range(N_CORES):
        xs = xT[:, c * S_LOC:(c + 1) * S_LOC]                     # (L, S_LOC)
        xTh = np.ascontiguousarray(
            xs.reshape(L // 128, 128, CH, CK).transpose(1, 2, 0, 3))
        in_maps.append({
            "xTh": xTh,
            "wkvTh": wkvTh,
            "qT": np.ascontiguousarray(qTf[:, c * QR:(c + 1) * QR]),
            "qres": np.ascontiguousarray(flat[c * QR:(c + 1) * QR, :]),
            "woutT": woutT,
            "maskb": mb,
            "gamma": gamma,
            "beta": beta,
        })
    return in_maps


def kernel(**inputs) -> np.ndarray:
    global _NC_CACHE
    in_maps = _make_in_maps(inputs)
    if _NC_CACHE is None:
        _NC_CACHE = build_program()
    res = run_bass_kernel_spmd(_NC_CACHE, in_maps, list(range(N_CORES)))
    out = np.concatenate([res.results[c]["y"] for c in range(N_CORES)], axis=0)
    return np.ascontiguousarray(out.reshape(J, Q, D).astype(np.float32))
